# revision 1
# baseline (speedup 1.0000x reference)
"""Deformable Transformer encoder layer on 8 Trainium2 NeuronCores (Bass/Tile).

Sharding: core k handles batch b=k//2, query half k%2 (2720 queries each);
the full layer runs per-core with no collectives, host stacks the slices.

Per-core dataflow (channel-on-partition "transposed" layout throughout):
  PE transposes src/pos/ref -> valueT (fp16) -> VP: interleaved sliding
  x-pairs so one packed f32 element = (v[x], v[x+1]) fp16 -> sampling
  offsets/attention logits via PE matmuls with biases folded in ->
  index + bilinear-weight pipeline on DVE/ACT in [(l,p,h), q] tiles
  (floor via int16 round trip with +1024 shift) -> idx wrap-transpose
  (strided converts + 3-dim partition-split DMAs) -> GPSIMD ap_gather
  -> combine: PE broadcasts weights over the 32 channels (sel-matmul),
  ACT drains to fp16, DVE multiplies, PE identity-matmuls accumulate
  pairs/points/levels in PSUM -> out-proj + LN + FFN + LN -> PE
  transpose back to row-major.

Self-contained: hardcodes all shapes; reads nothing from the problem dir.
"""
import sys
sys.path.insert(0, '/opt/trn_rl_repo')
import numpy as np
import ml_dtypes

import concourse.bass as bass
import concourse.mybir as mybir
import concourse.tile as tile
from concourse import bacc, library_config

f32 = mybir.dt.float32
f16 = mybir.dt.float16
i16 = mybir.dt.int16
AL = mybir.AluOpType
AF = mybir.ActivationFunctionType
AX = mybir.AxisListType

SPATIAL = [(64, 64), (32, 32), (16, 16), (8, 8)]
HWs = [h * w for h, w in SPATIAL]
LOFF = [0, 4096, 5120, 5376, 5440]
LEN, B, C, H, L, P, DH, DFF = 5440, 4, 256, 8, 4, 4, 32, 1024
NQ = 2720
EPS = 1e-5
NCHUNK = [512, 512, 512, 512, 512, 160]
COFF = [0, 512, 1024, 1536, 2048, 2560]
NQT = 22  # ceil(2720/128)
SH = 1024.0  # floor-trick shift
DEBUG = False
# Convert rounding differs between CoreSim (truncate toward zero) and HW
# (round-half-even). floor(px)+SH == trunc(px+SH) == rhe(px+SH-0.5), so the
# convert input needs +0.5 in sim mode only.
FLOOR_SIM = False


def _ceil(a, b):
    return (a + b - 1) // b


def build_nc():
    nc = bacc.Bacc(None, target_bir_lowering=False, debug=False)

    src_full_d = nc.dram_tensor("src_full", [LEN, C], f32, kind="ExternalInput")
    srcq_d = nc.dram_tensor("srcq", [NQ, C], f32, kind="ExternalInput")
    posq_d = nc.dram_tensor("posq", [NQ, C], f32, kind="ExternalInput")
    refq_d = nc.dram_tensor("refq", [NQ, 8], f32, kind="ExternalInput")
    w_val_d = nc.dram_tensor("w_val", [C, C], f32, kind="ExternalInput")
    bvalT_d = nc.dram_tensor("bvalT", [128, 2], f32, kind="ExternalInput")
    w_off_d = nc.dram_tensor("w_offp", [C, C], f32, kind="ExternalInput")
    refsel_d = nc.dram_tensor("refsel", [16, C], f32, kind="ExternalInput")
    w_attn_d = nc.dram_tensor("w_attnp", [C, 128], f32, kind="ExternalInput")
    b_attn_d = nc.dram_tensor("b_attnp", [1, 128], f32, kind="ExternalInput")
    w_out16_d = nc.dram_tensor("w_out16", [C, C], f32, kind="ExternalInput")
    boutT_d = nc.dram_tensor("boutT", [128, 2], f32, kind="ExternalInput")
    g1_d = nc.dram_tensor("g1T", [128, 2], f32, kind="ExternalInput")
    be1_d = nc.dram_tensor("be1T", [128, 2], f32, kind="ExternalInput")
    g2_d = nc.dram_tensor("g2T", [128, 2], f32, kind="ExternalInput")
    be2_d = nc.dram_tensor("be2T", [128, 2], f32, kind="ExternalInput")
    w1_d = nc.dram_tensor("w1", [C, DFF], f32, kind="ExternalInput")
    b1T_d = nc.dram_tensor("b1T", [128, 8], f32, kind="ExternalInput")
    w2_d = nc.dram_tensor("w2", [DFF, C], f32, kind="ExternalInput")
    b2T_d = nc.dram_tensor("b2T", [128, 2], f32, kind="ExternalInput")
    ident_d = nc.dram_tensor("ident", [128, 128], f32, kind="ExternalInput")
    ident16_d = nc.dram_tensor("ident16", [128, 128], f32, kind="ExternalInput")
    bsel16_d = nc.dram_tensor("bsel16", [128, 16, 128], f32, kind="ExternalInput")
    # per-(l,p,h)-partition consts:
    # 0: Wl, 1: SH+Wl-1, 2: SH+Wl-2, 3: SH+Hl-1, 4: SH*Wl+SH
    pc_d = nc.dram_tensor("pconst", [128, 5], f32, kind="ExternalInput")
    out_d = nc.dram_tensor("out", [NQ, C], f32, kind="ExternalOutput")
    if DEBUG:
        dbg_p5_d = nc.dram_tensor("dbg_p5", [2, 128, 512], f32,
                                  kind="ExternalOutput")
        dbg_aw_d = nc.dram_tensor("dbg_aw", [128, NQ], f16,
                                  kind="ExternalOutput")
        dbg_e_d = nc.dram_tensor("dbg_e", [2, 128, NQ], i16,
                                 kind="ExternalOutput")
        dbg_w4_d = nc.dram_tensor("dbg_w4", [128, 2, NQ, 2], f16,
                                  kind="ExternalOutput")
        dbg_acc_d = nc.dram_tensor("dbg_acc", [2, 128, NQ], f16,
                                   kind="ExternalOutput")
        dbg_x_d = nc.dram_tensor("dbg_x", [2, 128, NQ], f32,
                                 kind="ExternalOutput")
        dbg_g_d = nc.dram_tensor("dbg_g", [4, 128, 512], f32,
                                 kind="ExternalOutput")

    from contextlib import ExitStack
    with tile.TileContext(nc) as tc, ExitStack() as ctx:
        pool = lambda n, b: ctx.enter_context(tc.tile_pool(name=n, bufs=b))
        psum = lambda n, b: ctx.enter_context(
            tc.tile_pool(name=n, bufs=b, space="PSUM"))
        consts = pool("consts", 1)
        rowp = pool("rowp", 3)
        tp_ps = psum("tp_ps", 2)
        mm_ps = psum("mm_ps", 2)
        P_acc = tc.tile_pool(name="P_acc", bufs=1)
        p_acc = ctx.enter_context(P_acc)
        # LIFO-scoped pools: P_vp/P_w4 (long) under P_q/P_aw/P_ref (short)
        P_vp = tc.tile_pool(name="P_vp", bufs=1)
        p_vp = P_vp.__enter__()
        P_w4 = tc.tile_pool(name="P_w4", bufs=1)
        p_w4 = P_w4.__enter__()

        def cst(dram, shape, dtype=f32):
            t = consts.tile(shape, dtype, tag=dram.name + "_s", name=dram.name + "_s")
            nc.sync.dma_start(t[:], dram[:])
            return t

        def cstk(dram, nk, ncols, dtype=f32):
            ts = []
            for kb in range(nk):
                t = consts.tile([128, ncols], dtype,
                                tag=f"{dram.name}_k{kb}", name=f"{dram.name}_k{kb}")
                if dtype == f16:
                    for jc in range(_ceil(ncols, 512)):
                        a, bwid = jc * 512, min(512, ncols - jc * 512)
                        tmp = rowp.tile([128, 512], f32, tag="cvtw",
                                        name="cvtw")
                        nc.sync.dma_start(
                            tmp[:, :bwid],
                            dram[128 * kb:128 * kb + 128, a:a + bwid])
                        nc.vector.tensor_copy(t[:, a:a + bwid], tmp[:, :bwid])
                else:
                    nc.sync.dma_start(t[:], dram[128 * kb:128 * kb + 128])
                ts.append(t)
            return ts

        ident = cst(ident_d, [128, 128])
        ident16 = consts.tile([128, 128], f16, tag="ident16", name="ident16")
        nc.vector.tensor_copy(ident16[:], ident[:])
        bsel16 = consts.tile([128, 16, 128], f16, tag="bsel16", name="bsel16")
        bsv = bsel16[:].rearrange("p a b -> p (a b)")
        bdv = bsel16_d[:].rearrange("p a b -> p (a b)")
        for jc in range(4):
            a = jc * 512
            bstmp = rowp.tile([128, 512], f32, tag="cvtw", name="bstmp")
            nc.sync.dma_start(bstmp[:], bdv[:, a:a + 512])
            nc.vector.tensor_copy(bsv[:, a:a + 512], bstmp[:])
        pc = cst(pc_d, [128, 5])
        w_val = cstk(w_val_d, 2, C)
        bvalT = cst(bvalT_d, [128, 2])
        w_offp = cstk(w_off_d, 2, C, f16)
        refsel = cst(refsel_d, [16, C])
        w_attnp = cstk(w_attn_d, 2, 128, f16)
        b_attnp = cst(b_attn_d, [1, 128])
        w_out16 = cstk(w_out16_d, 2, C, f16)
        boutT = cst(boutT_d, [128, 2])
        g1T = cst(g1_d, [128, 2])
        be1T = cst(be1_d, [128, 2])
        g2T = cst(g2_d, [128, 2])
        be2T = cst(be2_d, [128, 2])
        b1T = cst(b1T_d, [128, 8])
        b2T = cst(b2T_d, [128, 2])

        ones_row = consts.tile([1, NQ], f32, tag="ones_row")
        nc.vector.memset(ones_row[:], 1.0)
        ones_col = consts.tile([128, 1], f32, tag="ones_col")
        nc.vector.memset(ones_col[:], 1.0)
        ones1x128 = consts.tile([1, 128], f32, tag="ones1x128")
        nc.vector.memset(ones1x128[:], 1.0)

        def mkconst(val, tag):
            t = consts.tile([128, 1], f32, tag=tag, name=tag)
            nc.vector.memset(t[:], val)
            return t

        c_zero = mkconst(0.0, "c_zero")
        c_eps1 = consts.tile([1, 1], f32, tag="c_eps1", name="c_eps1")
        nc.vector.memset(c_eps1[:], EPS)
        c_lo = mkconst(SH, "c_lo")          # shifted 0  (x0 >= 0 bound)
        c_lom1 = mkconst(SH - 1.0, "c_lom1")  # shifted -1 (x0 >= -1 bound)

        def bc(t, cn):
            return t[:, 0:1].to_broadcast([128, cn])

        def pcb(k, cn):
            return pc[:, k:k + 1].to_broadcast([128, cn])

        # ---------------- transposes ----------------
        def transpose_rows(dst_tiles, dram, nrows, add_dram=None):
            for i in range(_ceil(nrows, 128)):
                r0 = i * 128
                rn = min(128, nrows - r0)
                rt = rowp.tile([128, C], f32, tag="rows")
                nc.sync.dma_start(rt[:rn], dram[r0:r0 + rn])
                if add_dram is not None:
                    rt2 = rowp.tile([128, C], f32, tag="rows2")
                    nc.sync.dma_start(rt2[:rn], add_dram[r0:r0 + rn])
                    nc.vector.tensor_tensor(rt[:rn], rt[:rn], rt2[:rn],
                                            op=AL.add)
                for cb in range(2):
                    ps = tp_ps.tile([128, 128], f32, tag="tp")
                    nc.tensor.transpose(ps[:, :rn],
                                        rt[:rn, 128 * cb:128 * cb + 128],
                                        ident[:rn, :rn])
                    nc.scalar.copy(dst_tiles[cb][:, r0:r0 + rn], ps[:, :rn])

        # ---------------- valueT fp16 + VP interleaved pairs ----------------
        PH1 = tc.tile_pool(name="ph1", bufs=1)
        ph1 = PH1.__enter__()
        srcT = [ph1.tile([128, LEN], f32, tag=f"srcT{i}", name=f"srcT{i}")
                for i in range(2)]
        transpose_rows(srcT, src_full_d, LEN)
        v16 = [ph1.tile([128, LEN], f16, tag=f"v16_{i}", name=f"v16_{i}") for i in range(2)]
        for mb in range(2):
            for j in range(_ceil(LEN, 512)):
                c0 = j * 512
                cn = min(512, LEN - c0)
                ps = mm_ps.tile([128, 512], f32, tag="mm")
                for kb in range(2):
                    nc.tensor.matmul(ps[:, :cn],
                                     w_val[kb][:, 128 * mb:128 * mb + 128],
                                     srcT[kb][:, c0:c0 + cn],
                                     start=(kb == 0), stop=(kb == 1))
                nc.scalar.activation(v16[mb][:, c0:c0 + cn], ps[:, :cn],
                                     AF.Identity, bias=bvalT[:, mb:mb + 1])

        VP = [p_vp.tile([128, LEN], f32, tag=f"VP{g}", name=f"VP{g}") for g in range(2)]
        for g in range(2):
            vv = VP[g][:].bitcast(f16).rearrange("p (e s) -> p e s", s=2)
            nc.scalar.copy(vv[:, :, 0], v16[g][:])
            nc.vector.tensor_copy(vv[:, :LEN - 1, 1], v16[g][:, 1:])
            nc.vector.tensor_copy(vv[:, LEN - 1:, 1], v16[g][:, LEN - 1:])

        PH1.__exit__(None, None, None)

        # ---------------- qT / refT9 transposes ----------------
        P_q = tc.tile_pool(name="P_q", bufs=1)
        p_q = P_q.__enter__()
        P_aw = tc.tile_pool(name="P_aw", bufs=1)
        p_aw = P_aw.__enter__()
        P_ref = tc.tile_pool(name="P_ref", bufs=1)
        p_ref = P_ref.__enter__()
        qT = [p_q.tile([128, NQ], f16, tag=f"qT{i}", name=f"qT{i}")
              for i in range(2)]
        transpose_rows(qT, srcq_d, NQ, add_dram=posq_d)
        refT9 = p_ref.tile([16, NQ], f32, tag="refT9", name="refT9")
        nc.vector.memset(refT9[:], 1.0)
        for i in range(NQT):
            r0 = i * 128
            rn = min(128, NQ - r0)
            rt = rowp.tile([128, 8], f32, tag="refrows")
            nc.sync.dma_start(rt[:rn], refq_d[r0:r0 + rn])
            ps = tp_ps.tile([128, 128], f32, tag="tp")
            nc.tensor.transpose(ps[:8, :rn], rt[:rn, :8], ident[:rn, :rn])
            nc.scalar.copy(refT9[0:8, r0:r0 + rn], ps[:8, :rn])

        # ---------------- attention softmax -> awT ----------------
        awT = p_aw.tile([128, NQ], f16, tag="awT", name="awT")
        SMP = tc.tile_pool(name="smp", bufs=3)
        smp = SMP.__enter__()
        for i in range(NQT):
            r0 = i * 128
            rn = min(128, NQ - r0)
            ps = mm_ps.tile([128, 128], f32, tag="mm")
            for kb in range(2):
                nc.tensor.matmul(ps[:rn], qT[kb][:, r0:r0 + rn],
                                 w_attnp[kb][:],
                                 start=(kb == 0), stop=False)
            nc.tensor.matmul(ps[:rn], ones_row[:, r0:r0 + rn], b_attnp[:],
                             start=False, stop=True)
            aw = smp.tile([128, 128], f32, tag="aw")
            mx = smp.tile([128, 8], f32, tag="mx")
            sv = ps[:rn].rearrange("q (lp h) -> q h lp", h=8)
            av = aw[:rn].rearrange("q (lp h) -> q h lp", h=8)
            nc.vector.tensor_reduce(mx[:rn], sv, AX.X, op=AL.max)
            nc.vector.tensor_tensor(
                av, sv, mx[:rn].unsqueeze(2).to_broadcast([rn, 8, 16]),
                op=AL.subtract)
            nc.scalar.activation(aw[:rn], aw[:rn], AF.Exp)
            sm = smp.tile([128, 8], f32, tag="sm")
            nc.vector.tensor_reduce(sm[:rn], av, AX.X, op=AL.add)
            rc = smp.tile([128, 8], f32, tag="rc")
            nc.vector.reciprocal(rc[:rn], sm[:rn])
            nc.vector.tensor_tensor(
                av, av, rc[:rn].unsqueeze(2).to_broadcast([rn, 8, 16]),
                op=AL.mult)
            ps2 = tp_ps.tile([128, 128], f32, tag="tp")
            nc.tensor.transpose(ps2[:, :rn], aw[:rn], ident[:rn, :rn])
            nc.scalar.copy(awT[:, r0:r0 + rn], ps2[:, :rn])

        # ---------------- index/weight pipeline ----------------
        W4 = p_w4.tile([128, 2, NQ, 2], f16, tag="W4", name="W4")
        e16w = [p_w4.tile([128, 2, NQ], i16, tag=f"e16w{r}", name=f"e16w{r}") for r in range(2)]
        PIP = tc.tile_pool(name="pip", bufs=1)
        pip = PIP.__enter__()
        OFF_PS = tc.tile_pool(name="off_ps", bufs=1, space="PSUM")
        off_ps = OFF_PS.__enter__()

        for ci, cn in enumerate(NCHUNK):
            c0 = COFF[ci]
            pxy = []
            for comp in range(2):
                ps = off_ps.tile([128, 512], f32, tag=f"off{comp}")
                for kb in range(2):
                    nc.tensor.matmul(
                        ps[:, :cn],
                        w_offp[kb][:, 128 * comp:128 * comp + 128],
                        qT[kb][:, c0:c0 + cn], start=(kb == 0), stop=False)
                nc.tensor.matmul(ps[:, :cn],
                                 refsel[:, 128 * comp:128 * comp + 128],
                                 refT9[:, c0:c0 + cn], start=False, stop=True)
                if DEBUG and ci == 0:
                    dbg_t = pip.tile([128, 512], f32, tag="t1", name="dbgp5")
                    nc.scalar.copy(dbg_t[:], ps[:, :512])
                    nc.sync.dma_start(dbg_p5_d[comp], dbg_t[:])
                pxy.append(ps)
            p5x, p5y = pxy  # = coord - 0.5 + SH

            def T(tag, dtype=f32):
                return pip.tile([128, 512], dtype, tag=tag, name=tag)

            def axis_weights(p5, kb_hi1, kb_hi2):
                """z0f = floor(coord)+SH and frac weight wz1"""
                c16 = T("c16_" + kb_hi1, i16)
                if FLOOR_SIM:
                    ci_t = T("cvt_in")
                    nc.scalar.activation(ci_t[:, :cn], p5[:, :cn], AF.Copy,
                                         bias=0.5)
                    nc.vector.tensor_copy(c16[:, :cn], ci_t[:, :cn])
                else:
                    nc.vector.tensor_copy(c16[:, :cn], p5[:, :cn])
                z0f = T("z0f" + kb_hi1)
                nc.scalar.copy(z0f[:, :cn], c16[:, :cn])  # x0 + SH
                wz1 = T("wz1" + kb_hi1)
                nc.vector.scalar_tensor_tensor(wz1[:, :cn], p5[:, :cn], 0.5,
                                               z0f[:, :cn], op0=AL.add,
                                               op1=AL.subtract)
                return z0f, wz1

            # --- x ---
            x0f, wx1 = axis_weights(p5x, "x", None)
            wx0 = T("wx0")
            nc.scalar.activation(wx0[:, :cn], wx1[:, :cn], AF.Copy,
                                 bias=1.0, scale=-1.0)
            basex = T("basex")
            nc.vector.scalar_tensor_tensor(basex[:, :cn], x0f[:, :cn], SH,
                                           pcb(2, cn), op0=AL.max, op1=AL.min)
            dd = T("dd")
            nc.vector.tensor_tensor(dd[:, :cn], basex[:, :cn], x0f[:, :cn],
                                    op=AL.subtract)
            mA = T("mA")
            nc.scalar.activation(mA[:, :cn], dd[:, :cn], AF.Abs)
            nc.scalar.activation(mA[:, :cn], mA[:, :cn], AF.Copy,
                                 bias=1.0, scale=-1.0)
            mP = T("mP")
            nc.scalar.activation(mP[:, :cn], dd[:, :cn], AF.Relu)
            mM = T("mM")
            nc.scalar.activation(mM[:, :cn], dd[:, :cn], AF.Relu, scale=-1.0)
            t1 = T("t1")
            t2 = T("t2")
            wA_v = T("wA_v")
            nc.vector.tensor_tensor(t1[:, :cn], x0f[:, :cn], bc(c_lo, cn),
                                    op=AL.is_ge)
            nc.vector.tensor_tensor(wA_v[:, :cn], wx0[:, :cn], t1[:, :cn],
                                    op=AL.mult)
            nc.vector.tensor_tensor(t1[:, :cn], x0f[:, :cn], pcb(1, cn),
                                    op=AL.is_le)
            nc.vector.tensor_tensor(wA_v[:, :cn], wA_v[:, :cn], t1[:, :cn],
                                    op=AL.mult)
            wB_v = T("wB_v")
            nc.vector.tensor_tensor(t2[:, :cn], x0f[:, :cn], bc(c_lom1, cn),
                                    op=AL.is_ge)
            nc.vector.tensor_tensor(wB_v[:, :cn], wx1[:, :cn], t2[:, :cn],
                                    op=AL.mult)
            nc.vector.tensor_tensor(t2[:, :cn], x0f[:, :cn], pcb(2, cn),
                                    op=AL.is_le)
            nc.vector.tensor_tensor(wB_v[:, :cn], wB_v[:, :cn], t2[:, :cn],
                                    op=AL.mult)
            wsA = T("wsA")
            nc.vector.tensor_tensor(wsA[:, :cn], mA[:, :cn], wA_v[:, :cn],
                                    op=AL.mult)
            nc.vector.tensor_tensor(t1[:, :cn], mP[:, :cn], wB_v[:, :cn],
                                    op=AL.mult)
            nc.vector.tensor_tensor(wsA[:, :cn], wsA[:, :cn], t1[:, :cn],
                                    op=AL.add)
            wsB = T("wsB")
            nc.vector.tensor_tensor(wsB[:, :cn], mA[:, :cn], wB_v[:, :cn],
                                    op=AL.mult)
            nc.vector.tensor_tensor(t2[:, :cn], mM[:, :cn], wA_v[:, :cn],
                                    op=AL.mult)
            nc.vector.tensor_tensor(wsB[:, :cn], wsB[:, :cn], t2[:, :cn],
                                    op=AL.add)

            # --- y ---
            y0f, wy1 = axis_weights(p5y, "x", None)
            wy0 = T("wx0")
            nc.scalar.activation(wy0[:, :cn], wy1[:, :cn], AF.Copy,
                                 bias=1.0, scale=-1.0)
            yr0 = T("yr0")
            nc.vector.scalar_tensor_tensor(yr0[:, :cn], y0f[:, :cn], SH,
                                           pcb(3, cn), op0=AL.max, op1=AL.min)
            yr1 = T("yr1")
            nc.vector.scalar_tensor_tensor(yr1[:, :cn], y0f[:, :cn], 1.0,
                                           bc(c_lo, cn), op0=AL.add,
                                           op1=AL.max)
            nc.vector.tensor_tensor(yr1[:, :cn], yr1[:, :cn], pcb(3, cn),
                                    op=AL.min)
            wy0a = T("wA_v")
            nc.vector.tensor_tensor(t1[:, :cn], y0f[:, :cn], bc(c_lo, cn),
                                    op=AL.is_ge)
            nc.vector.tensor_tensor(wy0a[:, :cn], wy0[:, :cn], t1[:, :cn],
                                    op=AL.mult)
            nc.vector.tensor_tensor(t1[:, :cn], y0f[:, :cn], pcb(3, cn),
                                    op=AL.is_le)
            nc.vector.tensor_tensor(wy0a[:, :cn], wy0a[:, :cn], t1[:, :cn],
                                    op=AL.mult)
            nc.vector.tensor_tensor(wy0a[:, :cn], wy0a[:, :cn],
                                    awT[:, c0:c0 + cn], op=AL.mult)
            wy1a = T("wB_v")
            nc.vector.tensor_tensor(t2[:, :cn], y0f[:, :cn], bc(c_lom1, cn),
                                    op=AL.is_ge)
            nc.vector.tensor_tensor(wy1a[:, :cn], wy1[:, :cn], t2[:, :cn],
                                    op=AL.mult)
            # y0+1 <= Hl-1  <=>  y0f <= SH+Hl-2
            nc.vector.scalar_tensor_tensor(t2[:, :cn], pcb(3, cn), 1.0,
                                           y0f[:, :cn], op0=AL.subtract,
                                           op1=AL.is_ge)
            nc.vector.tensor_tensor(wy1a[:, :cn], wy1a[:, :cn], t2[:, :cn],
                                    op=AL.mult)
            nc.vector.tensor_tensor(wy1a[:, :cn], wy1a[:, :cn],
                                    awT[:, c0:c0 + cn], op=AL.mult)

            for (row, wya) in ((0, wy0a), (1, wy1a)):
                for (slot, wsx) in ((0, wsA), (1, wsB)):
                    nc.vector.tensor_tensor(
                        W4[:, row, c0:c0 + cn, slot], wsx[:, :cn],
                        wya[:, :cn], op=AL.mult)

            for row, yr in ((0, yr0), (1, yr1)):
                e = T("dd")
                nc.vector.scalar_tensor_tensor(e[:, :cn], yr[:, :cn], 0.0,
                                               pcb(0, cn), op0=AL.max,
                                               op1=AL.mult)
                nc.vector.tensor_tensor(e[:, :cn], e[:, :cn], basex[:, :cn],
                                        op=AL.add)
                nc.vector.tensor_tensor(e[:, :cn], e[:, :cn], pcb(4, cn),
                                        op=AL.subtract)
                ccols, cw0 = cn // 16, c0 // 16
                sv = e[:, :cn].rearrange("p (c w) -> p c w", w=16)
                for r in range(2):
                    dv = e16w[row][:, r].rearrange(
                        "p (w c) -> p c w",
                        c=NQ // 16)[:, cw0:cw0 + ccols, :]
                    nc.vector.tensor_copy(dv, sv)

        OFF_PS.__exit__(None, None, None)
        PIP.__exit__(None, None, None)
        SMP.__exit__(None, None, None)
        P_ref.__exit__(None, None, None)
        P_aw.__exit__(None, None, None)
        P_q.__exit__(None, None, None)

        if DEBUG:
            nc.sync.dma_start(dbg_aw_d[:], awT[:])
            for r in range(2):
                nc.sync.dma_start(dbg_e_d[r], e16w[r][:, 0, :])
            nc.sync.dma_start(dbg_w4_d[:], W4[:])

        # ---------------- wrap idx tiles ----------------
        nc.gpsimd.load_library(library_config.ap_gather)
        WRAPP = tc.tile_pool(name="wrapp", bufs=1)
        wrapp = WRAPP.__enter__()
        wraps = {}
        for l in range(L):
            for p in range(P):
                for row in range(2):
                    for g in range(2):
                        w = wrapp.tile([128, NQ // 16], i16,
                                       tag=f"wr{l}{p}{row}{g}",
                                       name=f"wr{l}{p}{row}{g}")
                        p0 = l * 32 + p * 8 + g * 4
                        src = e16w[row][p0:p0 + 4].rearrange(
                            "h r q -> h (r q)").rearrange(
                            "h (rw c) -> h rw c", c=NQ // 16)
                        nc.sync.dma_start(w[:], src)
                        wraps[(l, p, row, g)] = w

        # ---------------- gathers + combine ----------------
        GP = tc.tile_pool(name="gp", bufs=6)
        gp = GP.__enter__()
        WBP = tc.tile_pool(name="wbp", bufs=3)
        wbp = WBP.__enter__()
        MP = tc.tile_pool(name="mp", bufs=4)
        mp = MP.__enter__()
        ACC_PS = tc.tile_pool(name="acc_ps", bufs=1, space="PSUM")
        acc_ps = ACC_PS.__enter__()
        WB_PS = tc.tile_pool(name="wb_ps", bufs=1, space="PSUM")
        wb_ps = WB_PS.__enter__()
        accT16 = [p_acc.tile([128, NQ], f16, tag=f"accT16_{g}", name=f"accT16_{g}")
                  for g in range(2)]

        for ci, cn in enumerate(NCHUNK):
            c0 = COFF[ci]
            ccols, cw0 = cn // 16, c0 // 16
            for g in range(2):
                acc = acc_ps.tile([128, 512], f32, tag=f"acc{g}")
                n_mm = 0
                for l in range(L):
                    for p in range(P):
                        for row in range(2):
                            gt = gp.tile([128, 512], f32, tag="g")
                            nc.gpsimd.ap_gather(
                                gt[:, :cn],
                                VP[g][:, LOFF[l]:LOFF[l] + HWs[l]],
                                wraps[(l, p, row, g)][:, cw0:cw0 + ccols],
                                channels=128, num_elems=HWs[l], d=1,
                                num_idxs=cn)
                            if (DEBUG and ci == 0 and g == 0
                                    and l == 0 and p == 0 and row == 0):
                                nc.sync.dma_start(dbg_g_d[0], gt[:, :512])
                            if (DEBUG and ci == 0 and g == 0
                                    and l == 3 and p == 1 and row == 1):
                                nc.sync.dma_start(dbg_g_d[1], gt[:, :512])
                            wb = wb_ps.tile([128, 1024], f32, tag="wb")
                            b64 = l // 2
                            si = (l % 2) * 8 + p * 2 + g
                            sel = bsel16[64 * b64:64 * b64 + 64, si, :]
                            rsrc = W4[64 * b64:64 * b64 + 64, row,
                                      c0:c0 + cn, :] \
                                .rearrange("h q s -> h (q s)")
                            nc.tensor.matmul(wb[:, :cn], sel,
                                             rsrc[:, :cn],
                                             start=True, stop=True)
                            nc.tensor.matmul(wb[:, cn:2 * cn], sel,
                                             rsrc[:, cn:2 * cn],
                                             start=True, stop=True)
                            wb16 = wbp.tile([128, 1024], f16, tag="wb16")
                            nc.scalar.copy(wb16[:, :2 * cn], wb[:, :2 * cn])
                            m = mp.tile([128, 1024], f16, tag="m")
                            nc.vector.tensor_tensor(
                                m[:, :2 * cn],
                                gt[:, :cn].bitcast(f16),
                                wb16[:, :2 * cn], op=AL.mult)
                            mv = m[:, :2 * cn].rearrange(
                                "p (q s) -> p q s", s=2)
                            last = (l == L - 1 and p == P - 1 and row == 1)
                            nc.tensor.matmul(acc[:, :cn], ident16[:],
                                             mv[:, :, 0],
                                             start=(n_mm == 0), stop=False)
                            nc.tensor.matmul(acc[:, :cn], ident16[:],
                                             mv[:, :, 1],
                                             start=False, stop=last)
                            n_mm += 2
                nc.scalar.copy(accT16[g][:, c0:c0 + cn], acc[:, :cn])

        WB_PS.__exit__(None, None, None)
        ACC_PS.__exit__(None, None, None)
        MP.__exit__(None, None, None)
        WBP.__exit__(None, None, None)
        GP.__exit__(None, None, None)
        WRAPP.__exit__(None, None, None)
        P_w4.__exit__(None, None, None)
        P_vp.__exit__(None, None, None)

        if DEBUG:
            for g_ in range(2):
                nc.sync.dma_start(dbg_acc_d[g_], accT16[g_][:])

        # ---------------- out-proj + residual + LN1 ----------------
        p_f = ctx.enter_context(tc.tile_pool(name="P_f", bufs=1))
        srcqT = [p_f.tile([128, NQ], f32, tag=f"srcqT{i}", name=f"srcqT{i}")
                 for i in range(2)]
        transpose_rows(srcqT, srcq_d, NQ)
        w1 = cstk(w1_d, 2, DFF, f16)
        w2 = cstk(w2_d, 8, C, f16)
        lnp = pool("lnp", 1)
        ln_ps = psum("ln_ps", 1)

        def layernorm_T(xT, gT, beT, dstT):
            for j in range(_ceil(NQ, 512)):
                c0j, cnj = j * 512, min(512, NQ - j * 512)
                psm = ln_ps.tile([1, 512], f32, tag="lnm", name="lnm")
                psv = ln_ps.tile([1, 512], f32, tag="lnv", name="lnv")
                sqc = [None, None]
                for i in range(2):
                    sqc[i] = lnp.tile([128, 512], f32, tag=f"sqc{i}",
                                      name=f"sqc{i}")
                    nc.vector.tensor_tensor(sqc[i][:, :cnj],
                                            xT[i][:, c0j:c0j + cnj],
                                            xT[i][:, c0j:c0j + cnj],
                                            op=AL.mult)
                for i in range(2):
                    nc.tensor.matmul(psm[:, :cnj], ones_col[:],
                                     xT[i][:, c0j:c0j + cnj],
                                     start=(i == 0), stop=(i == 1))
                for i in range(2):
                    nc.tensor.matmul(psv[:, :cnj], ones_col[:],
                                     sqc[i][:, :cnj],
                                     start=(i == 0), stop=(i == 1))
                mrow = lnp.tile([1, 512], f32, tag="mrow", name="mrow")
                vrow = lnp.tile([1, 512], f32, tag="vrow", name="vrow")
                nc.scalar.activation(mrow[:, :cnj], psm[:, :cnj], AF.Copy,
                                     scale=1.0 / C)
                nc.scalar.activation(vrow[:, :cnj], psv[:, :cnj], AF.Copy,
                                     scale=1.0 / C)
                msq = lnp.tile([1, 512], f32, tag="msq", name="msq")
                nc.vector.tensor_tensor(msq[:, :cnj], mrow[:, :cnj],
                                        mrow[:, :cnj], op=AL.mult)
                nc.vector.tensor_tensor(vrow[:, :cnj], vrow[:, :cnj],
                                        msq[:, :cnj], op=AL.subtract)
                nc.scalar.activation(vrow[:, :cnj], vrow[:, :cnj], AF.Sqrt,
                                     bias=c_eps1[:])
                rrow = lnp.tile([1, 512], f32, tag="rrow", name="rrow")
                nc.vector.reciprocal(rrow[:, :cnj], vrow[:, :cnj])
                psbm = ln_ps.tile([128, 512], f32, tag="lnbm", name="lnbm")
                psbr = ln_ps.tile([128, 512], f32, tag="lnbr", name="lnbr")
                nc.tensor.matmul(psbm[:, :cnj], ones1x128[:],
                                 mrow[:, :cnj], start=True, stop=True)
                nc.tensor.matmul(psbr[:, :cnj], ones1x128[:],
                                 rrow[:, :cnj], start=True, stop=True)
                for i in range(2):
                    t = lnp.tile([128, 512], f32, tag="lt", name="lt")
                    nc.vector.tensor_tensor(t[:, :cnj], xT[i][:, c0j:c0j + cnj],
                                            psbm[:, :cnj], op=AL.subtract)
                    nc.vector.tensor_tensor(t[:, :cnj], t[:, :cnj],
                                            psbr[:, :cnj], op=AL.mult)
                    nc.vector.scalar_tensor_tensor(
                        dstT[i][:, c0j:c0j + cnj], t[:, :cnj], gT[:, i:i + 1],
                        beT[:, i:i + 1].to_broadcast([128, cnj]),
                        op0=AL.mult, op1=AL.add)

        xT = [p_f.tile([128, NQ], f32, tag=f"xT{i}", name=f"xT{i}") for i in range(2)]
        pre = [lnp.tile([128, NQ], f32, tag=f"pre{i}", name=f"pre{i}") for i in range(2)]
        for mb in range(2):
            for j in range(_ceil(NQ, 512)):
                c0j, cnj = j * 512, min(512, NQ - j * 512)
                ps = mm_ps.tile([128, 512], f32, tag="mm")
                for kb in range(2):
                    nc.tensor.matmul(ps[:, :cnj],
                                     w_out16[kb][:, 128 * mb:128 * mb + 128],
                                     accT16[kb][:, c0j:c0j + cnj],
                                     start=(kb == 0), stop=(kb == 1))
                nc.scalar.activation(pre[mb][:, c0j:c0j + cnj], ps[:, :cnj],
                                     AF.Identity, bias=boutT[:, mb:mb + 1])
        for i in range(2):
            nc.vector.tensor_tensor(pre[i][:], pre[i][:], srcqT[i][:],
                                    op=AL.add)
        layernorm_T(pre, g1T, be1T, xT)

        if DEBUG:
            for i in range(2):
                nc.sync.dma_start(dbg_x_d[i], xT[i][:])

        # ---------------- FFN ----------------
        xT16 = [p_f.tile([128, NQ], f16, tag=f"xT16_{i}", name=f"xT16_{i}")
                for i in range(2)]
        for i in range(2):
            nc.vector.tensor_copy(xT16[i][:], xT[i][:])
        fpre = [lnp.tile([128, NQ], f32, tag=f"pre{i}", name=f"fpre{i}") for i in range(2)]
        hp = ctx.enter_context(tc.tile_pool(name="hp", bufs=2))
        for j in range(_ceil(NQ, 512)):
            c0j, cnj = j * 512, min(512, NQ - j * 512)
            hts = []
            for mb in range(8):
                ps = mm_ps.tile([128, 512], f32, tag="mm")
                for kb in range(2):
                    nc.tensor.matmul(ps[:, :cnj],
                                     w1[kb][:, 128 * mb:128 * mb + 128],
                                     xT16[kb][:, c0j:c0j + cnj],
                                     start=(kb == 0), stop=(kb == 1))
                ht = hp.tile([128, 512], f16, tag=f"ht{mb}", name=f"ht{mb}")
                nc.scalar.activation(ht[:, :cnj], ps[:, :cnj],
                                     AF.Relu, bias=b1T[:, mb:mb + 1])
                hts.append(ht)
            for mb in range(2):
                ps = mm_ps.tile([128, 512], f32, tag="mm")
                for kb in range(8):
                    nc.tensor.matmul(ps[:, :cnj],
                                     w2[kb][:, 128 * mb:128 * mb + 128],
                                     hts[kb][:, :cnj],
                                     start=(kb == 0), stop=(kb == 7))
                nc.scalar.activation(fpre[mb][:, c0j:c0j + cnj], ps[:, :cnj],
                                     AF.Identity, bias=b2T[:, mb:mb + 1])
        outT = [p_f.tile([128, NQ], f32, tag=f"outT{i}", name=f"outT{i}") for i in range(2)]
        for i in range(2):
            nc.vector.tensor_tensor(fpre[i][:], fpre[i][:], xT[i][:],
                                    op=AL.add)
        layernorm_T(fpre, g2T, be2T, outT)

        # ---------------- final transpose + store ----------------
        for i in range(NQT):
            r0 = i * 128
            rn = min(128, NQ - r0)
            ot = rowp.tile([128, C], f32, tag="orow")
            for cb in range(2):
                ps = tp_ps.tile([128, 128], f32, tag="tp")
                nc.tensor.transpose(ps[:rn], outT[cb][:, r0:r0 + rn], ident[:])
                nc.scalar.copy(ot[:rn, 128 * cb:128 * cb + 128], ps[:rn])
            nc.sync.dma_start(out_d[r0:r0 + rn], ot[:rn])

    nc.compile()
    return nc


def build_baseline_nc():
    """Same I/O signature, trivial work - for dispatch-overhead baseline."""
    nc = bacc.Bacc(None, target_bir_lowering=False, debug=False)
    ds = {}
    ds['src_full'] = nc.dram_tensor("src_full", [LEN, C], f32, kind="ExternalInput")
    ds['srcq'] = nc.dram_tensor("srcq", [NQ, C], f32, kind="ExternalInput")
    ds['posq'] = nc.dram_tensor("posq", [NQ, C], f32, kind="ExternalInput")
    ds['refq'] = nc.dram_tensor("refq", [NQ, 8], f32, kind="ExternalInput")
    ds['w_val'] = nc.dram_tensor("w_val", [C, C], f32, kind="ExternalInput")
    ds['bvalT'] = nc.dram_tensor("bvalT", [128, 2], f32, kind="ExternalInput")
    ds['w_offp'] = nc.dram_tensor("w_offp", [C, C], f32, kind="ExternalInput")
    ds['refsel'] = nc.dram_tensor("refsel", [16, C], f32, kind="ExternalInput")
    ds['w_attnp'] = nc.dram_tensor("w_attnp", [C, 128], f32, kind="ExternalInput")
    ds['b_attnp'] = nc.dram_tensor("b_attnp", [1, 128], f32, kind="ExternalInput")
    ds['w_out16'] = nc.dram_tensor("w_out16", [C, C], f32, kind="ExternalInput")
    ds['boutT'] = nc.dram_tensor("boutT", [128, 2], f32, kind="ExternalInput")
    ds['g1T'] = nc.dram_tensor("g1T", [128, 2], f32, kind="ExternalInput")
    ds['be1T'] = nc.dram_tensor("be1T", [128, 2], f32, kind="ExternalInput")
    ds['g2T'] = nc.dram_tensor("g2T", [128, 2], f32, kind="ExternalInput")
    ds['be2T'] = nc.dram_tensor("be2T", [128, 2], f32, kind="ExternalInput")
    ds['w1'] = nc.dram_tensor("w1", [C, DFF], f32, kind="ExternalInput")
    ds['b1T'] = nc.dram_tensor("b1T", [128, 8], f32, kind="ExternalInput")
    ds['w2'] = nc.dram_tensor("w2", [DFF, C], f32, kind="ExternalInput")
    ds['b2T'] = nc.dram_tensor("b2T", [128, 2], f32, kind="ExternalInput")
    ds['ident'] = nc.dram_tensor("ident", [128, 128], f32, kind="ExternalInput")
    ds['ident16'] = nc.dram_tensor("ident16", [128, 128], f32, kind="ExternalInput")
    ds['bsel16'] = nc.dram_tensor("bsel16", [128, 16, 128], f32, kind="ExternalInput")
    ds['pconst'] = nc.dram_tensor("pconst", [128, 5], f32, kind="ExternalInput")
    out_d = nc.dram_tensor("out", [NQ, C], f32, kind="ExternalOutput")
    if DEBUG:
        dbg_p5_d = nc.dram_tensor("dbg_p5", [2, 128, 512], f32,
                                  kind="ExternalOutput")
        dbg_aw_d = nc.dram_tensor("dbg_aw", [128, NQ], f16,
                                  kind="ExternalOutput")
        dbg_e_d = nc.dram_tensor("dbg_e", [2, 128, NQ], i16,
                                 kind="ExternalOutput")
        dbg_w4_d = nc.dram_tensor("dbg_w4", [128, 2, NQ, 2], f16,
                                  kind="ExternalOutput")
        dbg_acc_d = nc.dram_tensor("dbg_acc", [2, 128, NQ], f16,
                                   kind="ExternalOutput")
        dbg_x_d = nc.dram_tensor("dbg_x", [2, 128, NQ], f32,
                                 kind="ExternalOutput")
        dbg_g_d = nc.dram_tensor("dbg_g", [4, 128, 512], f32,
                                 kind="ExternalOutput")
    with tile.TileContext(nc) as tc:
        with tc.tile_pool(name="p", bufs=2) as pl:
            for i in range(_ceil(NQ, 128)):
                r0 = i * 128
                rn = min(128, NQ - r0)
                t = pl.tile([128, C], f32, tag="t", name="t")
                nc.sync.dma_start(t[:rn], ds['srcq'][r0:r0 + rn])
                nc.sync.dma_start(out_d[r0:r0 + rn], t[:rn])
    nc.compile()
    return nc


# ======================= host side =======================

def _mk_bsel16():
    b = np.zeros((128, 16, 128), np.float32)
    for l2 in range(2):
        for p in range(4):
            for g in range(2):
                si = l2 * 8 + p * 2 + g
                for h2 in range(4):
                    k = l2 * 32 + p * 8 + 4 * g + h2
                    b[k, si, 32 * h2:32 * h2 + 32] = 1.0
                    b[64 + k, si, 32 * h2:32 * h2 + 32] = 1.0
    return b


def host_prep(inputs):
    """Build the 8 per-core input maps from full inputs."""
    src = np.asarray(inputs['src'], np.float32)
    pos = np.asarray(inputs['pos'], np.float32)
    ref = np.asarray(inputs['reference_points'], np.float32)
    vr = np.asarray(inputs['valid_ratios'], np.float32)

    # reference: loc = ref[:,:,None,l,None,:] * (valid_ratios==1 here) + ...
    # fold valid_ratios into refsel? reference multiplies ref by valid_ratios
    # only when reference_points has L dim... (see reference: loc = ref + off/norm;
    # valid_ratios enters as ones). We fold vr=1 assumption but keep general:
    # scale per (b, l): refq scaled host-side.
    refs = ref * vr[:, None, :, :]          # [B, Len, L, 2]

    co = lambda h, l, p, c: ((c * L + l) * P + p) * 8 + (h)  # noqa

    # permuted column order m = comp*128 + l*32 + p*8 + h
    w_off = np.asarray(inputs['w_off'], np.float32)
    b_off = np.asarray(inputs['b_off'], np.float32)
    w_attn = np.asarray(inputs['w_attn'], np.float32)
    b_attn = np.asarray(inputs['b_attn'], np.float32)
    perm_off = np.zeros(256, np.int64)
    for comp in range(2):
        for l in range(L):
            for p in range(P):
                for h in range(H):
                    m = comp * 128 + l * 32 + p * 8 + h
                    perm_off[m] = ((h * L + l) * P + p) * 2 + comp
    w_offp = w_off[:, perm_off].copy()
    b_offp = b_off[perm_off].copy()
    perm_attn = np.zeros(128, np.int64)
    for l in range(L):
        for p in range(P):
            for h in range(H):
                perm_attn[l * 32 + p * 8 + h] = (h * L + l) * P + p
    w_attnp = w_attn[:, perm_attn].copy()
    b_attnp = b_attn[perm_attn].reshape(1, 128).copy()

    # refsel [16, 256]: rows j=(l*2+comp) -> grid scale; row 8 -> ones coeff
    refsel = np.zeros((16, 256), np.float32)
    for comp in range(2):
        for l in range(L):
            Hl, Wl = SPATIAL[l]
            norm = Wl if comp == 0 else Hl
            for p in range(P):
                for h in range(H):
                    m = comp * 128 + l * 32 + p * 8 + h
                    refsel[l * 2 + comp, m] = float(norm)
    refsel[8, :] = b_offp - 1.0 + SH

    pconst = np.zeros((128, 5), np.float32)
    for l in range(L):
        Hl, Wl = SPATIAL[l]
        for p in range(P):
            for h in range(H):
                r = l * 32 + p * 8 + h
                pconst[r] = [Wl, SH + Wl - 1, SH + Wl - 2, SH + Hl - 1,
                             SH * Wl + SH]

    def t2(v):
        return np.ascontiguousarray(
            v.reshape(2, 128).T.astype(np.float32))

    common = {
        'w_val': np.asarray(inputs['w_val'], np.float32),
        'bvalT': t2(np.asarray(inputs['b_val'], np.float32)),
        'w_offp': w_offp, 'refsel': refsel,
        'w_attnp': w_attnp, 'b_attnp': b_attnp,
        'w_out16': np.asarray(inputs['w_out'], np.float32),
        'boutT': t2(np.asarray(inputs['b_out'], np.float32)),
        'g1T': t2(np.asarray(inputs['g1'], np.float32)),
        'be1T': t2(np.asarray(inputs['be1'], np.float32)),
        'g2T': t2(np.asarray(inputs['g2'], np.float32)),
        'be2T': t2(np.asarray(inputs['be2'], np.float32)),
        'w1': np.asarray(inputs['w1'], np.float32),
        'b1T': np.ascontiguousarray(
            np.asarray(inputs['b1'], np.float32).reshape(8, 128).T),
        'w2': np.asarray(inputs['w2'], np.float32),
        'b2T': t2(np.asarray(inputs['b2'], np.float32)),
        'ident': np.eye(128, dtype=np.float32),
        'ident16': np.eye(128, dtype=np.float32),
        'bsel16': _mk_bsel16(),
        'pconst': pconst,
    }
    in_maps = []
    for core in range(8):
        b, half = core // 2, core % 2
        q0 = half * NQ
        im = dict(common)
        im['src_full'] = np.ascontiguousarray(src[b])
        im['srcq'] = np.ascontiguousarray(src[b, q0:q0 + NQ])
        im['posq'] = np.ascontiguousarray(pos[b, q0:q0 + NQ])
        im['refq'] = np.ascontiguousarray(
            refs[b, q0:q0 + NQ].reshape(NQ, 8))
        in_maps.append(im)
    return in_maps


_CACHE = {}


def _get_runner():
    if 'run' in _CACHE:
        return _CACHE['run']
    import jax
    from jax.sharding import Mesh, PartitionSpec
    from jax.experimental.shard_map import shard_map
    from concourse.bass2jax import (_bass_exec_p, install_neuronx_cc_hook,
                                    partition_id_tensor)
    nc = build_nc()
    _CACHE['nc'] = nc
    install_neuronx_cc_hook()
    partition_name = (nc.partition_id_tensor.name
                      if nc.partition_id_tensor else None)
    in_names, out_names, out_avals = [], [], []
    for alloc in nc.m.functions[0].allocations:
        if not isinstance(alloc, mybir.MemoryLocationSet):
            continue
        name = alloc.memorylocations[0].name
        if alloc.kind == "ExternalInput":
            if name != partition_name:
                in_names.append(name)
        elif alloc.kind == "ExternalOutput":
            out_names.append(name)
            out_avals.append(jax.core.ShapedArray(
                tuple(alloc.tensor_shape), mybir.dt.np(alloc.dtype)))
    n_params = len(in_names)
    n_outs = len(out_avals)
    zero_outs = [np.zeros(a.shape, a.dtype) for a in out_avals]
    all_names = list(in_names) + out_names
    if partition_name is not None:
        all_names.append(partition_name)
    donate = tuple(range(n_params, n_params + n_outs))

    def _body(*args):
        operands = list(args)
        if partition_name is not None:
            operands.append(partition_id_tensor())
        outs = _bass_exec_p.bind(
            *operands, out_avals=tuple(out_avals), in_names=tuple(all_names),
            out_names=tuple(out_names), lowering_input_output_aliases=(),
            sim_require_finite=True, sim_require_nnan=True, nc=nc)
        return tuple(outs)

    devices = jax.devices()[:8]
    mesh = Mesh(np.asarray(devices), ("core",))
    jit = jax.jit(shard_map(_body, mesh=mesh,
                            in_specs=(PartitionSpec("core"),) * (n_params + n_outs),
                            out_specs=(PartitionSpec("core"),) * n_outs,
                            check_rep=False),
                  donate_argnums=donate, keep_unused=True)

    def run(in_maps):
        args = [np.concatenate([np.asarray(m[n]) for m in in_maps], axis=0)
                for n in in_names]
        args += [np.concatenate([z.copy() for _ in range(8)], axis=0)
                 for z in zero_outs]
        outs = jit(*args)
        res = [dict() for _ in range(8)]
        for n, o in zip(out_names, outs):
            o = np.asarray(o)
            per = o.shape[0] // 8
            for c in range(8):
                res[c][n] = o[c * per:(c + 1) * per]
        return res

    _CACHE['run'] = run
    return run


def kernel(**inputs):
    in_maps = host_prep(inputs)
    run = _get_runner()
    res = run(in_maps)
    out = np.zeros((B, LEN, C), np.float32)
    for core in range(8):
        b, half = core // 2, core % 2
        out[b, half * NQ:(half + 1) * NQ] = res[core]['out']
    # int32 preservation n/a: output is f32
    return out

